# revision 12
# baseline (speedup 1.0000x reference)
"""Trainium2 Bass kernel for GNN NodeProcessor (segment_sum + MLP + LayerNorm + residual).

Strategy (8 NeuronCores, SPMD, no collectives):
  - Host: bucket edges by destination node-tile (128 nodes per tile). Assign
    node tiles to cores rank-matched by block count so one static per-position
    block schedule covers all 8 cores. Each core receives only the edges that
    target its own node tiles, pre-permuted and zero-padded to blocks of 128
    edges, plus local destination indices (j mod 128) per edge slot.
  - Device phase A (aggregation): for each node tile, accumulate
        aggT[f, n] += edge_block[e, f].T @ onehot[e, n]
    in PSUM, where onehot[e, n] = (j_loc[e] == n) is built on-device from an
    iota row via a DVE tensor_scalar is_equal against a per-partition scalar.
  - Device phase B (MLP + LN): in [feat, node] layout,
        h1 = silu(W1.T-chunks @ [xT; aggT] + b1), h2 = W2-chunks @ h1 + b2,
    transpose back to [node, feat] with TensorE, LayerNorm via bn_stats,
    gamma/beta/residual, DMA out.
  - Host: reassemble node tiles into the full [N, D] output.
"""

import numpy as np

P = 128
D = 128
N_CORES = 8


class Cfg:
    def __init__(self, n_nodes, tpc, dt_edge="float16", sg=16, gt=4, eps=1e-5,
                 native_silu=False):
        self.n_nodes = n_nodes
        self.tpc = tpc              # node tiles per core
        self.nt = N_CORES * tpc     # global node tiles (padded)
        self.dt_edge = dt_edge
        self.sg = sg                # superblocks per edge DMA batch
        self.gt = gt                # node tiles per MLP group
        self.eps = eps
        self.native_silu = native_silu


REAL_CFG = Cfg(n_nodes=50000, tpc=50)

LAST_RESULTS = None  # BassKernelResults of the most recent run (for test.py)


# ---------------------------------------------------------------- host prep

def _prepare(x, edge_index, edge_attr, cfg):
    np_edge = np.dtype(cfg.dt_edge)
    j = np.asarray(edge_index)[1].astype(np.int64)
    gtile = j >> 7  # j // 128
    cnt = np.bincount(gtile, minlength=cfg.nt)
    bpt = np.maximum((cnt + P - 1) // P, 1)  # blocks per tile (>=1)

    order = np.argsort(-bpt, kind="stable")
    gmap = order[: cfg.nt].reshape(cfg.tpc, N_CORES).T  # [core, pos] global tile id
    schedule = [int(bpt[order[k * N_CORES]]) for k in range(cfg.tpc)]
    if sum(schedule) % 2:
        schedule[-1] += 1
    nb = sum(schedule)
    base = np.concatenate([[0], np.cumsum(schedule)])

    eorder = np.argsort(gtile, kind="stable")
    tstart = np.concatenate([[0], np.cumsum(cnt)])

    ea = np.asarray(edge_attr)
    x = np.asarray(x, dtype=np.float32)

    shards = []
    node_ids_all = []
    for c in range(N_CORES):
        attr_pad = np.zeros((nb * P, D), dtype=np_edge)
        jl = np.zeros((nb * P,), dtype=np.float32)
        for k in range(cfg.tpc):
            gt_id = int(gmap[c, k])
            s0 = int(tstart[gt_id])
            csz = int(cnt[gt_id])
            if csz == 0:
                continue
            rows = eorder[s0 : s0 + csz]
            dst0 = int(base[k]) * P
            attr_pad[dst0 : dst0 + csz] = ea[rows]
            jl[dst0 : dst0 + csz] = (j[rows] - gt_id * P).astype(np.float32)
        nsb = nb // 2
        edges_pair = (
            attr_pad.reshape(nsb, 2, P, D).transpose(0, 2, 1, 3).reshape(nsb * P, 2 * D)
        )
        jlT = np.ascontiguousarray(jl.reshape(nb, P).T)

        node_ids = (gmap[c][:, None] * P + np.arange(P)[None, :]).reshape(-1)
        valid = node_ids < cfg.n_nodes
        xs = x[np.minimum(node_ids, cfg.n_nodes - 1)] * valid[:, None].astype(np.float32)
        xt = np.ascontiguousarray(xs.T)

        shards.append({"edges": edges_pair, "jloc": jlT, "xs": xs, "xt": xt})
        node_ids_all.append(node_ids)

    return schedule, shards, node_ids_all


# ---------------------------------------------------------------- device build

def _build(cfg, schedule):
    import concourse.bacc as bacc
    import concourse.bass as bass
    import concourse.mybir as mybir
    import concourse.tile as tile
    from concourse.masks import make_identity

    dt = mybir.dt
    f32 = dt.float32
    dte = getattr(dt, cfg.dt_edge)
    Af = mybir.ActivationFunctionType
    Op = mybir.AluOpType

    TPC = cfg.tpc
    NPC = TPC * P          # padded nodes per core
    NB = sum(schedule)     # total edge blocks per core
    NSB = NB // 2          # superblocks (block pairs)
    SG = cfg.sg
    GT = cfg.gt

    nc = bacc.Bacc(None)
    edges_d = nc.dram_tensor("edges", [NSB * P, 2 * D], dte, kind="ExternalInput")
    jloc_d = nc.dram_tensor("jloc", [P, NB], f32, kind="ExternalInput")
    xs_d = nc.dram_tensor("xs", [NPC, D], f32, kind="ExternalInput")
    xt_d = nc.dram_tensor("xt", [D, NPC], f32, kind="ExternalInput")
    w1_d = nc.dram_tensor("w1", [2 * D, 2 * D], f32, kind="ExternalInput")
    w2_d = nc.dram_tensor("w2", [2 * D, D], f32, kind="ExternalInput")
    b1_d = nc.dram_tensor("b1", [2 * D, 1], f32, kind="ExternalInput")
    b2_d = nc.dram_tensor("b2", [D, 1], f32, kind="ExternalInput")
    gam_d = nc.dram_tensor("gam", [P, D], f32, kind="ExternalInput")
    bet_d = nc.dram_tensor("bet", [P, D], f32, kind="ExternalInput")
    out_d = nc.dram_tensor("out", [NPC, D], f32, kind="ExternalOutput")

    with tile.TileContext(nc) as tc:
        with (
            tc.tile_pool(name="const", bufs=1) as cpool,
            tc.tile_pool(name="ebatch", bufs=4) as epool,
            tc.tile_pool(name="oh", bufs=28) as ohpool,
            tc.tile_pool(name="agg", bufs=3) as aggpool,
            tc.tile_pool(name="h1s", bufs=3) as h1pool,
            tc.tile_pool(name="h2s", bufs=3) as h2pool,
            tc.tile_pool(name="ln", bufs=8) as lnpool,
            tc.tile_pool(name="stat", bufs=16) as stpool,
            tc.tile_pool(name="ost", bufs=4) as ospool,
            tc.tile_pool(name="psA", bufs=2, space="PSUM") as psA,
            tc.tile_pool(name="psB", bufs=1, space="PSUM") as psB,
            tc.tile_pool(name="psT", bufs=2, space="PSUM") as psT,
        ):
            # ---- constants
            iota_i = cpool.tile([P, P], dt.int32)
            nc.gpsimd.iota(iota_i[:], pattern=[[1, P]], base=0, channel_multiplier=0)
            iota_f = cpool.tile([P, P], f32)
            nc.vector.tensor_copy(iota_f[:], iota_i[:])
            if cfg.dt_edge == "float32":
                iota_e = iota_f
            else:
                iota_e = cpool.tile([P, P], dte)
                nc.vector.tensor_copy(iota_e[:], iota_f[:])

            ident = cpool.tile([P, P], f32)
            make_identity(nc, ident[:])

            w1_sb = cpool.tile([P, 2, 2 * D], f32, tag="w1")
            nc.sync.dma_start(out=w1_sb[:, 0, :], in_=w1_d[0:P, :])
            nc.sync.dma_start(out=w1_sb[:, 1, :], in_=w1_d[P : 2 * P, :])
            w2_sb = cpool.tile([P, 2, D], f32, tag="w2")
            nc.sync.dma_start(out=w2_sb[:, 0, :], in_=w2_d[0:P, :])
            nc.sync.dma_start(out=w2_sb[:, 1, :], in_=w2_d[P : 2 * P, :])
            b1_sb = cpool.tile([P, 2], f32, tag="b1")
            nc.sync.dma_start(out=b1_sb[:, 0:1], in_=b1_d[0:P, :])
            nc.sync.dma_start(out=b1_sb[:, 1:2], in_=b1_d[P : 2 * P, :])
            b2_sb = cpool.tile([P, 1], f32, tag="b2")
            nc.sync.dma_start(out=b2_sb[:], in_=b2_d[:, :])
            gam_sb = cpool.tile([P, D], f32, tag="gam")
            nc.sync.dma_start(out=gam_sb[:], in_=gam_d[:, :])
            bet_sb = cpool.tile([P, D], f32, tag="bet")
            nc.sync.dma_start(out=bet_sb[:], in_=bet_d[:, :])
            eps_sb = cpool.tile([P, 1], f32, tag="eps")
            nc.vector.memset(eps_sb[:], cfg.eps)

            jl_sb = cpool.tile([P, NB], f32, tag="jl")
            nc.sync.dma_start(out=jl_sb[:], in_=jloc_d[:, :])

            # whole x in [node-in-tile, tile*feat] layout and xT in [feat, node]
            xs_sb = cpool.tile([P, TPC * D], f32, tag="xs")
            nc.sync.dma_start(
                out=xs_sb[:].rearrange("p (t f) -> p t f", f=D),
                in_=xs_d[:, :].rearrange("(t p) f -> p t f", p=P),
            )
            xt_sb = cpool.tile([P, NPC], f32, tag="xt")
            nc.sync.dma_start(out=xt_sb[:], in_=xt_d[:, :])

            # ---- main loop
            groups = []
            k0 = 0
            while k0 < TPC:
                gts = min(GT, TPC - k0)
                groups.append((k0, gts))
                k0 += gts

            n_batches = (NSB + SG - 1) // SG
            blk = 0
            cur_batch = -1
            batch_tile = None
            batch_cols = 0

            for (k0, gts) in groups:
                gw = gts * P
                agg_g = aggpool.tile([P, GT * P], f32, tag="agg")
                for q in range(gts):
                    k = k0 + q
                    ps = psA.tile([P, P], f32, tag="psA")
                    nblk = schedule[k]
                    for i in range(nblk):
                        s, par = divmod(blk, 2)
                        bi = s // SG
                        if bi != cur_batch:
                            nsb_here = min(SG, NSB - bi * SG)
                            batch_cols = nsb_here * 2 * D
                            batch_tile = epool.tile([P, SG * 2 * D], dte, tag="eb")
                            src = edges_d[
                                bi * SG * P : bi * SG * P + nsb_here * P, :
                            ].rearrange("(s p) f -> p s f", p=P)
                            nc.sync.dma_start(
                                out=batch_tile[:, :batch_cols].rearrange(
                                    "p (s f) -> p s f", f=2 * D
                                ),
                                in_=src,
                            )
                            cur_batch = bi
                        col = (s % SG) * 2 * D + par * D
                        oh = ohpool.tile([P, P], dte, tag="oh")
                        nc.vector.tensor_scalar(
                            out=oh[:],
                            in0=iota_e[:],
                            scalar1=jl_sb[:, blk : blk + 1],
                            scalar2=None,
                            op0=Op.is_equal,
                        )
                        nc.tensor.matmul(
                            out=ps[:],
                            lhsT=batch_tile[:, col : col + D],
                            rhs=oh[:],
                            start=(i == 0),
                            stop=(i == nblk - 1),
                        )
                        blk += 1
                    nc.scalar.copy(agg_g[:, q * P : (q + 1) * P], ps[:])

                # ---- MLP for this group of gts node tiles
                nsl = slice(k0 * P, k0 * P + gw)
                h1a_ps = psB.tile([P, GT * P], f32, tag="h1a")
                h1b_ps = psB.tile([P, GT * P], f32, tag="h1b")
                nc.tensor.matmul(
                    out=h1a_ps[:, :gw], lhsT=w1_sb[:, 0, 0:P], rhs=xt_sb[:, nsl],
                    start=True, stop=False,
                )
                nc.tensor.matmul(
                    out=h1a_ps[:, :gw], lhsT=w1_sb[:, 1, 0:P], rhs=agg_g[:, :gw],
                    start=False, stop=True,
                )
                nc.tensor.matmul(
                    out=h1b_ps[:, :gw], lhsT=w1_sb[:, 0, P : 2 * P], rhs=xt_sb[:, nsl],
                    start=True, stop=False,
                )
                nc.tensor.matmul(
                    out=h1b_ps[:, :gw], lhsT=w1_sb[:, 1, P : 2 * P], rhs=agg_g[:, :gw],
                    start=False, stop=True,
                )
                h1a_sb = h1pool.tile([P, GT * P], f32, tag="h1as")
                h1b_sb = h1pool.tile([P, GT * P], f32, tag="h1bs")
                if cfg.native_silu:
                    nc.scalar.activation(
                        h1a_sb[:, :gw], h1a_ps[:, :gw], Af.Silu, bias=b1_sb[:, 0:1]
                    )
                    nc.scalar.activation(
                        h1b_sb[:, :gw], h1b_ps[:, :gw], Af.Silu, bias=b1_sb[:, 1:2]
                    )
                else:
                    # silu(z) = z * sigmoid(z), z = h1 + b1
                    for (hps, hsb, bsl) in (
                        (h1a_ps, h1a_sb, b1_sb[:, 0:1]),
                        (h1b_ps, h1b_sb, b1_sb[:, 1:2]),
                    ):
                        zpre = h1pool.tile([P, GT * P], f32, tag="zpre")
                        nc.scalar.activation(
                            zpre[:, :gw], hps[:, :gw], Af.Identity, bias=bsl
                        )
                        zsig = h1pool.tile([P, GT * P], f32, tag="zsig")
                        nc.scalar.activation(
                            zsig[:, :gw], hps[:, :gw], Af.Sigmoid, bias=bsl
                        )
                        nc.vector.tensor_tensor(
                            out=hsb[:, :gw], in0=zpre[:, :gw], in1=zsig[:, :gw],
                            op=Op.mult,
                        )
                h2_ps = psB.tile([P, GT * P], f32, tag="h2")
                nc.tensor.matmul(
                    out=h2_ps[:, :gw], lhsT=w2_sb[:, 0, :], rhs=h1a_sb[:, :gw],
                    start=True, stop=False,
                )
                nc.tensor.matmul(
                    out=h2_ps[:, :gw], lhsT=w2_sb[:, 1, :], rhs=h1b_sb[:, :gw],
                    start=False, stop=True,
                )
                h2_sb = h2pool.tile([P, GT * P], f32, tag="h2s")
                nc.scalar.activation(
                    h2_sb[:, :gw], h2_ps[:, :gw], Af.Identity, bias=b2_sb[:, :]
                )

                ost = ospool.tile([P, GT * P], f32, tag="ost")
                for q in range(gts):
                    k = k0 + q
                    tp = psT.tile([P, P], f32, tag="tp")
                    nc.tensor.transpose(
                        tp[:], h2_sb[:, q * P : (q + 1) * P], ident[:]
                    )
                    # LayerNorm over feat (free dim) for 128 nodes
                    stats = stpool.tile([P, 6], f32, tag="bst")
                    nc.vector.bn_stats(out=stats[:], in_=tp[:])
                    mv = stpool.tile([P, 2], f32, tag="mv")
                    nc.vector.bn_aggr(out=mv[:], in_=stats[:])
                    sd = stpool.tile([P, 1], f32, tag="sd")
                    nc.scalar.activation(
                        sd[:], mv[:, 1:2], Af.Sqrt, bias=eps_sb[:, :]
                    )
                    rstd = stpool.tile([P, 1], f32, tag="rstd")
                    nc.vector.reciprocal(rstd[:], sd[:])
                    t1 = lnpool.tile([P, P], f32, tag="t1")
                    nc.vector.tensor_scalar(
                        out=t1[:], in0=tp[:],
                        scalar1=mv[:, 0:1], scalar2=rstd[:, :],
                        op0=Op.subtract, op1=Op.mult,
                    )
                    t2 = lnpool.tile([P, P], f32, tag="t2")
                    nc.vector.tensor_tensor(
                        out=t2[:], in0=t1[:], in1=gam_sb[:], op=Op.mult
                    )
                    xpb = lnpool.tile([P, P], f32, tag="xpb")
                    nc.gpsimd.tensor_tensor(
                        out=xpb[:], in0=xs_sb[:, k * P : (k + 1) * P],
                        in1=bet_sb[:], op=Op.add,
                    )
                    nc.vector.tensor_tensor(
                        out=ost[:, q * P : (q + 1) * P], in0=t2[:], in1=xpb[:],
                        op=Op.add,
                    )
                nc.sync.dma_start(
                    out=out_d[k0 * P : k0 * P + gw, :].rearrange(
                        "(t p) f -> p t f", p=P
                    ),
                    in_=ost[:, :gw].rearrange("p (t f) -> p t f", f=D),
                )
    nc.finalize()
    return nc


# ---------------------------------------------------------------- run

def _run(inputs, cfg, use_sim=False):
    global LAST_RESULTS
    x = np.asarray(inputs["x"], dtype=np.float32)
    W1 = np.asarray(inputs["W1"], dtype=np.float32)
    b1 = np.asarray(inputs["b1"], dtype=np.float32).reshape(2 * D, 1)
    W2 = np.asarray(inputs["W2"], dtype=np.float32)
    b2 = np.asarray(inputs["b2"], dtype=np.float32).reshape(D, 1)
    gam = np.tile(np.asarray(inputs["ln_gamma"], dtype=np.float32), (P, 1))
    bet = np.tile(np.asarray(inputs["ln_beta"], dtype=np.float32), (P, 1))

    schedule, shards, node_ids = _prepare(
        x, inputs["edge_index"], inputs["edge_attr"], cfg
    )
    nc = _build(cfg, schedule)

    common = {"w1": W1, "w2": W2, "b1": b1, "b2": b2, "gam": gam, "bet": bet}
    in_maps = [dict(sh, **common) for sh in shards]

    if use_sim:
        from concourse import bass_interp

        outs = []
        for c in range(N_CORES):
            sim = bass_interp.MultiCoreSim(nc, 1)
            for name, arr in in_maps[c].items():
                sim.cores[0].tensor(name)[:] = arr
            sim.simulate()
            outs.append(np.array(sim.cores[0].mem_tensor("out")))
    else:
        from concourse.bass_utils import run_bass_kernel_spmd

        res = run_bass_kernel_spmd(nc, in_maps, list(range(N_CORES)))
        LAST_RESULTS = res
        outs = [res.results[c]["out"] for c in range(N_CORES)]

    out_full = np.zeros((cfg.nt * P, D), dtype=np.float32)
    for c in range(N_CORES):
        out_full[node_ids[c]] = outs[c]
    return out_full[: cfg.n_nodes]


def kernel(**inputs):
    return _run(inputs, REAL_CFG, use_sim=False)
